# revision 18
# baseline (speedup 1.0000x reference)
"""Trainium2 Bass kernel: 8-head MHA (dense_transformer), batch-sharded on 8 cores.

Per-core (batch b) dataflow, all matmuls fp32r (full-rate, FP22 mantissa):
  phase 1: QT = Wq @ query^T   [D, Lq]  (head h = partition-block h)
           KT = Wk @ keys^T    [D, Lk]
           V  = keys @ Wv^T    [Lk, D]  (natural)
  phase 2, per (q-chunk 512, head):
           S       = QT_h^T @ KT_h            (PSUM, natural [q, k])
           attn    = S + Z                     (DVE;  Z = rel masked w/ -3.2e10)
           P       = exp(attn/32), rowsum      (ACT, fused accumulate)
           P_norm  = P * (1/rowsum)            (DVE tensor_scalar, 2x mode)
           sim[h]  <- P_norm                   (DMA out, natural layout)
           P^T     = PE-transpose(P_norm)      (128x128 tiles -> PSUM -> SBUF)
           out^T  += V_h-block^T-form matmuls  (lhsT=V chunk, rhs=P^T, N=512)
           outT    <- out^T                    (DMA out; host transposes back)

Host side: shards batch across cores, pre-transposes query/keys/W (layout
marshaling), gathers and re-assembles the full outputs.
"""

import numpy as np

import concourse.bass as bass
from concourse import bacc
import concourse.mybir as mybir
import concourse.tile as tile
from concourse.bass_utils import run_bass_kernel_spmd
from concourse.masks import make_identity

B = 8
H = 8
L = 1024  # Lq == Lk
D = 1024
DH = D // H  # 128

F32 = mybir.dt.float32
F32R = mybir.dt.float32r
BF16 = mybir.dt.bfloat16
I32 = mybir.dt.int32

# Masked logits become -3.2e10/32 = -1e9 inside the exp scale -> exp() == 0.
NEG_MASK = -3.2e10
INV_SQRT_D = 1.0 / 32.0  # 1/sqrt(D)


def r(ap):
    """fp32r view of an fp32 AP (same bits, full-rate PE matmul)."""
    return ap.bitcast(F32R)


def build_nc(trace_scopes: bool = False) -> bass.Bass:
    nc = bacc.Bacc(
        trn_type="TRN2",
        target_bir_lowering=False,
        debug=False,
        enable_asserts=False,
        num_devices=B,
    )

    qT = nc.dram_tensor("qT", [D, L], F32R, kind="ExternalInput")  # query[b].T
    kT = nc.dram_tensor("kT", [D, L], F32R, kind="ExternalInput")  # keys[b].T
    msk = nc.dram_tensor("msk", [L, L], I32, kind="ExternalInput")  # mask[b]
    rel = nc.dram_tensor("rel", [L, L], F32, kind="ExternalInput")  # rel_emb[b]
    wqT = nc.dram_tensor("wqT", [D, D], F32R, kind="ExternalInput")  # Wq.T
    wkT = nc.dram_tensor("wkT", [D, D], F32R, kind="ExternalInput")  # Wk.T
    wvT = nc.dram_tensor("wvT", [D, D], F32R, kind="ExternalInput")  # Wv.T
    outT = nc.dram_tensor("outT", [D, L], F32, kind="ExternalOutput")  # out[b].T
    sim = nc.dram_tensor("sim", [H, L, L], F32, kind="ExternalOutput")

    # DRAM views with 128-partition tiling: "(t p) x -> p t x"
    qT_t = qT.rearrange("(t p) x -> p t x", p=128)
    kT_t = kT.rearrange("(t p) x -> p t x", p=128)
    wqT_t = wqT.rearrange("(t p) x -> p t x", p=128)
    wkT_t = wkT.rearrange("(t p) x -> p t x", p=128)
    wvT_t = wvT.rearrange("(t p) x -> p t x", p=128)

    with tile.TileContext(nc) as tc:
        with tc.tile_pool(name="persist", bufs=1) as persist:
            # Persistent phase-2 operands (96.5 KB/partition).
            qtp = persist.tile([128, H, L], F32R, tag="qtp")  # Q^T
            ktp = persist.tile([128, H, L], F32R, tag="ktp")  # K^T
            vsb = persist.tile([128, H, L], F32R, tag="vsb")  # V natural [kk, j]
            ident = persist.tile([128, 128], F32R, tag="ident")
            ident_src = persist.tile([128, 128], F32, tag="ident_src")
            make_identity(nc, ident_src)
            nc.vector.tensor_copy(ident, ident_src)

            # Whole-kernel PSUM pools: projections borrow the attention
            # S-pool, so there is no pool-transition barrier between phases.
            with (
                tc.tile_pool(name="s_psum", bufs=2, space="PSUM") as s_psum,
                tc.tile_pool(name="t_psum", bufs=2, space="PSUM") as t_psum,
                tc.tile_pool(name="o_psum", bufs=2, space="PSUM") as o_psum,
                tc.tile_pool(name="stats", bufs=8) as stats,
            ):
                # k-tile-chunked loads: fine-grained deps so the first
                # matmuls start after ~1 chunk instead of a full 4 MB load.
                def chunk_load(pool, tag, dram_t, n=8, bufs=8):
                    tiles = []
                    for t in range(n):
                        ct = pool.tile([128, 1, L], F32R, tag=tag, bufs=bufs)
                        nc.sync.dma_start(out=ct, in_=dram_t[:, t : t + 1, :])
                        tiles.append(ct)
                    return tiles

                def proj_group(lhsT_of_kt, rhs_of_kt_half, out_1024):
                    ps = s_psum.tile([128, L], F32, tag="s")
                    for half in range(2):
                        for kt in range(8):
                            nc.tensor.matmul(
                                ps[:, half * 512 : (half + 1) * 512],
                                lhsT_of_kt(kt),
                                rhs_of_kt_half(kt, half),
                                start=(kt == 0),
                                stop=(kt == 7),
                            )
                    nc.vector.tensor_copy(out_1024, ps)

                win = tc.alloc_tile_pool(name="win", bufs=1)
                qtpool = tc.alloc_tile_pool(name="qtpool", bufs=1)
                ktpool = tc.alloc_tile_pool(name="ktpool", bufs=1)
                if True:
                    wk_in, kt_in = [], []
                    for t in range(8):
                        wt = win.tile([128, 1, L], F32R, tag="w", bufs=11)
                        nc.sync.dma_start(out=wt, in_=wkT_t[:, t : t + 1, :])
                        wk_in.append(wt)
                        ct = ktpool.tile([128, 1, L], F32R, tag="kt_in", bufs=8)
                        nc.sync.dma_start(out=ct, in_=kT_t[:, t : t + 1, :])
                        kt_in.append(ct)

                    # K projection: KT[j, kk] ; lhsT = wkT chunk, rhs = kT chunk
                    for jb in range(H):
                        proj_group(
                            lambda kt, jb=jb: wk_in[kt][:, 0, jb * 128 : (jb + 1) * 128],
                            lambda kt, hf: kt_in[kt][:, 0, hf * 512 : (hf + 1) * 512],
                            ktp[:, jb, :],
                        )

                    # V projection: V[kk, j] ; lhsT = kT chunk, rhs = wvT chunk
                    wv_in = chunk_load(win, "w", wvT_t, bufs=11)
                    qt_in = chunk_load(qtpool, "qt_in", qT_t)
                    for kb in range(8):
                        proj_group(
                            lambda kt, kb=kb: kt_in[kt][:, 0, kb * 128 : (kb + 1) * 128],
                            lambda kt, hf: wv_in[kt][:, 0, hf * 512 : (hf + 1) * 512],
                            vsb[:, kb, :],
                        )

                # ktpool closed: Z fits; prep overlaps the Q projection.
                ktpool.release()
                zpool = tc.alloc_tile_pool(name="zpool", bufs=1, side="right")
                zsb = zpool.tile([128, H, L], BF16, tag="zsb")
                with tc.tile_pool(name="zstage", bufs=1) as zstage:
                    nc.gpsimd.memset(zsb, NEG_MASK)
                    for qi in range(8):
                        mt = zstage.tile([128, L], I32, tag="mt", bufs=1)
                        rt = zstage.tile([128, L], F32, tag="rt", bufs=1)
                        nc.sync.dma_start(out=mt, in_=msk[qi * 128 : (qi + 1) * 128, :])
                        nc.sync.dma_start(out=rt, in_=rel[qi * 128 : (qi + 1) * 128, :])
                        nc.vector.copy_predicated(out=zsb[:, qi, :], mask=mt, data=rt)

                    # Q projection: QT[j, q] ; lhsT = wqT chunk, rhs = qT chunk
                    wq_in = chunk_load(win, "w", wqT_t, bufs=11)
                    for jb in range(H):
                        proj_group(
                            lambda kt, jb=jb: wq_in[kt][:, 0, jb * 128 : (jb + 1) * 128],
                            lambda kt, hf: qt_in[kt][:, 0, hf * 512 : (hf + 1) * 512],
                            qtp[:, jb, :],
                        )

                # win/qtpool space is recycled for the attention work pool.
                qtpool.release()
                win.release()
                with tc.tile_pool(name="work", bufs=1) as work:
                  for qc in range(2):
                    for h in range(H):
                        pT = work.tile([128, 8, 512], F32R, tag="pT", bufs=2)
                        for qt4 in range(4):
                            qi = qc * 4 + qt4
                            qs = slice(qi * 128, (qi + 1) * 128)

                            s = s_psum.tile([128, L], F32, tag="s")
                            for kc in range(2):
                                nc.tensor.matmul(
                                    s[:, kc * 512 : (kc + 1) * 512],
                                    qtp[:, h, qs],
                                    ktp[:, h, kc * 512 : (kc + 1) * 512],
                                    start=True,
                                    stop=True,
                                )
                            a = work.tile([128, L], F32R, tag="a", bufs=8)
                            nc.vector.tensor_add(a, s, zsb[:, qi, :])
                            rs = stats.tile([128, 1], F32, tag="rs", bufs=12)
                            nc.scalar.activation(
                                a,
                                a,
                                mybir.ActivationFunctionType.Exp,
                                scale=INV_SQRT_D,
                                accum_out=rs,
                            )
                            rec = stats.tile([128, 1], F32, tag="rec", bufs=12)
                            nc.vector.reciprocal(rec, rs)
                            nc.gpsimd.tensor_scalar_mul(a, a, rec)
                            nc.sync.dma_start(out=sim[h, qs, :], in_=a.bitcast(F32))

                            # P^T via PE transpose, batched 4 tiles per PSUM bank
                            for half in range(2):
                                tp = t_psum.tile([128, 512], F32R, tag="tp")
                                for j in range(4):
                                    kb = half * 4 + j
                                    nc.tensor.transpose(
                                        tp[:, j * 128 : (j + 1) * 128],
                                        a[:, kb * 128 : (kb + 1) * 128],
                                        ident,
                                    )
                                dst = pT[
                                    :, half * 4 : half * 4 + 4, qt4 * 128 : (qt4 + 1) * 128
                                ]
                                src = tp.rearrange("p (t x) -> p t x", t=4)
                                if half == 0:
                                    nc.vector.tensor_copy(dst, src)
                                else:
                                    nc.scalar.copy(dst, src)

                        # PV: out^T[j, q] accumulated over kk blocks (N=512)
                        ot = o_psum.tile([128, 512], F32, tag="ot")
                        for kb in range(8):
                            nc.tensor.matmul(
                                ot,
                                vsb[:, kb, h * 128 : (h + 1) * 128],
                                pT[:, kb, :],
                                start=(kb == 0),
                                stop=(kb == 7),
                            )
                        ots = work.tile([128, 512], F32, tag="ots", bufs=2)
                        nc.scalar.copy(ots, ot)
                        nc.sync.dma_start(
                            out=outT[h * 128 : (h + 1) * 128, qc * 512 : (qc + 1) * 512],
                            in_=ots,
                        )
                zpool.release()
    nc.finalize()
    return nc


_NC_CACHE = None


def _get_nc():
    global _NC_CACHE
    if _NC_CACHE is None:
        _NC_CACHE = build_nc()
    return _NC_CACHE


def kernel(query, keys, mask, relative_embedding, Wq, Wk, Wv, _trace=False, _nc=None):
    query = np.asarray(query, dtype=np.float32)
    keys = np.asarray(keys, dtype=np.float32)
    mask = np.asarray(mask, dtype=np.int32)
    rel = np.asarray(relative_embedding, dtype=np.float32)
    wqT = np.ascontiguousarray(np.asarray(Wq, dtype=np.float32).T)
    wkT = np.ascontiguousarray(np.asarray(Wk, dtype=np.float32).T)
    wvT = np.ascontiguousarray(np.asarray(Wv, dtype=np.float32).T)

    nc = _nc if _nc is not None else _get_nc()

    in_maps = []
    for b in range(B):
        in_maps.append(
            {
                "qT": np.ascontiguousarray(query[b].T),
                "kT": np.ascontiguousarray(keys[b].T),
                "msk": np.ascontiguousarray(mask[b]),
                "rel": np.ascontiguousarray(rel[b]),
                "wqT": wqT,
                "wkT": wkT,
                "wvT": wvT,
            }
        )

    res = run_bass_kernel_spmd(
        nc,
        in_maps,
        core_ids=list(range(B)),
        trace=_trace,
    )

    output = np.empty((B, L, D), np.float32)
    similarity = np.empty((H * B, L, L), np.float32)
    for b in range(B):
        output[b] = res.results[b]["outT"].T
        similarity[b::B] = res.results[b]["sim"]

    if _trace:
        kernel._last_results = res
    return output, similarity


# revision 24
# speedup vs baseline: 39.5866x; 39.5866x over previous
"""Trainium2 Bass kernel: 8-head MHA (dense_transformer), batch-sharded on 8 cores.

Per-core (batch b) dataflow, all matmuls fp32r (full-rate, FP22 mantissa):
  phase 1: QT = Wq @ query^T   [D, Lq]  (head h = partition-block h)
           KT = Wk @ keys^T    [D, Lk]
           V  = keys @ Wv^T    [Lk, D]  (natural)
  phase 2, per (q-chunk 512, head):
           S       = QT_h^T @ KT_h            (PSUM, natural [q, k])
           attn    = S + Z                     (DVE;  Z = rel masked w/ -3.2e10)
           P       = exp(attn/32), rowsum      (ACT, fused accumulate)
           P_norm  = P * (1/rowsum)            (DVE tensor_scalar, 2x mode)
           sim[h]  <- P_norm                   (DMA out, natural layout)
           P^T     = PE-transpose(P_norm)      (128x128 tiles -> PSUM -> SBUF)
           out^T  += V_h-block^T-form matmuls  (lhsT=V chunk, rhs=P^T, N=512)
           outT    <- out^T                    (DMA out; host transposes back)

Host side: shards batch across cores, pre-transposes query/keys/W (layout
marshaling), gathers and re-assembles the full outputs.
"""

import numpy as np

import concourse.bass as bass
from concourse import bacc
import concourse.mybir as mybir
import concourse.tile as tile
from concourse.bass_utils import run_bass_kernel_spmd
from concourse.masks import make_identity

B = 8
H = 8
L = 1024  # Lq == Lk
D = 1024
DH = D // H  # 128

F32 = mybir.dt.float32
F32R = mybir.dt.float32r
BF16 = mybir.dt.bfloat16
I32 = mybir.dt.int32

# Masked logits become -3.2e10/32 = -1e9 inside the exp scale -> exp() == 0.
NEG_MASK = -3.2e10
INV_SQRT_D = 1.0 / 32.0  # 1/sqrt(D)


def r(ap):
    """fp32r view of an fp32 AP (same bits, full-rate PE matmul)."""
    return ap.bitcast(F32R)


def build_nc(trace_scopes: bool = False) -> bass.Bass:
    nc = bacc.Bacc(
        trn_type="TRN2",
        target_bir_lowering=False,
        debug=False,
        enable_asserts=False,
        num_devices=B,
    )

    qT = nc.dram_tensor("qT", [D, L], F32R, kind="ExternalInput")  # query[b].T
    kT = nc.dram_tensor("kT", [D, L], F32R, kind="ExternalInput")  # keys[b].T
    msk = nc.dram_tensor("msk", [L, L], I32, kind="ExternalInput")  # mask[b]
    rel = nc.dram_tensor("rel", [L, L], F32, kind="ExternalInput")  # rel_emb[b]
    wqT = nc.dram_tensor("wqT", [D, D], F32R, kind="ExternalInput")  # Wq.T
    wkT = nc.dram_tensor("wkT", [D, D], F32R, kind="ExternalInput")  # Wk.T
    wvT = nc.dram_tensor("wvT", [D, D], F32R, kind="ExternalInput")  # Wv.T
    outT = nc.dram_tensor("outT", [D, L], F32, kind="ExternalOutput")  # out[b].T
    sim = nc.dram_tensor("sim", [H, L, L], F32, kind="ExternalOutput")

    # DRAM views with 128-partition tiling: "(t p) x -> p t x"
    qT_t = qT.rearrange("(t p) x -> p t x", p=128)
    kT_t = kT.rearrange("(t p) x -> p t x", p=128)
    wqT_t = wqT.rearrange("(t p) x -> p t x", p=128)
    wkT_t = wkT.rearrange("(t p) x -> p t x", p=128)
    wvT_t = wvT.rearrange("(t p) x -> p t x", p=128)

    with tile.TileContext(nc) as tc:
        with tc.tile_pool(name="persist", bufs=1) as persist:
            # Persistent phase-2 operands (96.5 KB/partition).
            qtp = persist.tile([128, H, L], F32R, tag="qtp")  # Q^T
            ktp = persist.tile([128, H, L], F32R, tag="ktp")  # K^T
            vsb = persist.tile([128, H, L], F32R, tag="vsb")  # V natural [kk, j]
            ident = persist.tile([128, 128], F32R, tag="ident")
            ident_src = persist.tile([128, 128], F32, tag="ident_src")
            make_identity(nc, ident_src)
            nc.vector.tensor_copy(ident, ident_src)

            # Whole-kernel PSUM pools: projections borrow the attention
            # S-pool, so there is no pool-transition barrier between phases.
            with (
                tc.tile_pool(name="s_psum", bufs=2, space="PSUM") as s_psum,
                tc.tile_pool(name="t_psum", bufs=2, space="PSUM") as t_psum,
                tc.tile_pool(name="o_psum", bufs=2, space="PSUM") as o_psum,
                tc.tile_pool(name="stats", bufs=8) as stats,
            ):
                # k-tile-chunked loads: fine-grained deps so the first
                # matmuls start after ~1 chunk instead of a full 4 MB load.
                def chunk_load(pool, tag, dram_t, n=8, bufs=8):
                    tiles = []
                    for t in range(n):
                        ct = pool.tile([128, 1, L], F32R, tag=tag, bufs=bufs)
                        nc.sync.dma_start(out=ct, in_=dram_t[:, t : t + 1, :])
                        tiles.append(ct)
                    return tiles

                def proj_group(lhsT_of_kt, rhs_of_kt_half, out_1024):
                    ps = s_psum.tile([128, L], F32, tag="s")
                    for half in range(2):
                        for kt in range(8):
                            nc.tensor.matmul(
                                ps[:, half * 512 : (half + 1) * 512],
                                lhsT_of_kt(kt),
                                rhs_of_kt_half(kt, half),
                                start=(kt == 0),
                                stop=(kt == 7),
                            )
                    nc.scalar.copy(out_1024, ps)

                win = tc.alloc_tile_pool(name="win", bufs=1)
                qtpool = tc.alloc_tile_pool(name="qtpool", bufs=1)
                ktpool = tc.alloc_tile_pool(name="ktpool", bufs=1)
                if True:
                    wk_in, kt_in = [], []
                    for t in range(8):
                        wt = win.tile([128, 1, L], F32R, tag="w", bufs=11)
                        nc.sync.dma_start(out=wt, in_=wkT_t[:, t : t + 1, :])
                        wk_in.append(wt)
                        ct = ktpool.tile([128, 1, L], F32R, tag="kt_in", bufs=8)
                        nc.sync.dma_start(out=ct, in_=kT_t[:, t : t + 1, :])
                        kt_in.append(ct)

                    # K projection: KT[j, kk] ; lhsT = wkT chunk, rhs = kT chunk
                    for jb in range(H):
                        proj_group(
                            lambda kt, jb=jb: wk_in[kt][:, 0, jb * 128 : (jb + 1) * 128],
                            lambda kt, hf: kt_in[kt][:, 0, hf * 512 : (hf + 1) * 512],
                            ktp[:, jb, :],
                        )

                    # V projection: V[kk, j] ; lhsT = kT chunk, rhs = wvT chunk
                    wv_in = chunk_load(win, "w", wvT_t, bufs=11)
                    qt_in = chunk_load(qtpool, "qt_in", qT_t)
                    for kb in range(8):
                        proj_group(
                            lambda kt, kb=kb: kt_in[kt][:, 0, kb * 128 : (kb + 1) * 128],
                            lambda kt, hf: wv_in[kt][:, 0, hf * 512 : (hf + 1) * 512],
                            vsb[:, kb, :],
                        )

                # ktpool closed: Z fits; prep overlaps the Q projection.
                ktpool.release()
                zpool = tc.alloc_tile_pool(name="zpool", bufs=1, side="right")
                zsb = zpool.tile([128, H, L], BF16, tag="zsb")
                with tc.tile_pool(name="zstage", bufs=1) as zstage:
                    nc.vector.memset(zsb, NEG_MASK)
                    for qi in range(8):
                        mt = zstage.tile([128, L], I32, tag="mt", bufs=1)
                        rt = zstage.tile([128, L], F32, tag="rt", bufs=1)
                        nc.sync.dma_start(out=mt, in_=msk[qi * 128 : (qi + 1) * 128, :])
                        nc.sync.dma_start(out=rt, in_=rel[qi * 128 : (qi + 1) * 128, :])
                        nc.vector.copy_predicated(out=zsb[:, qi, :], mask=mt, data=rt)

                    # Q projection: QT[j, q] ; lhsT = wqT chunk, rhs = qT chunk
                    wq_in = chunk_load(win, "w", wqT_t, bufs=11)
                    for jb in range(H):
                        proj_group(
                            lambda kt, jb=jb: wq_in[kt][:, 0, jb * 128 : (jb + 1) * 128],
                            lambda kt, hf: qt_in[kt][:, 0, hf * 512 : (hf + 1) * 512],
                            qtp[:, jb, :],
                        )

                # win/qtpool space is recycled for the attention work pool.
                qtpool.release()
                win.release()
                with tc.tile_pool(name="work", bufs=1) as work:
                  for qc in range(2):
                    for h in range(H):
                        pT = work.tile([128, 8, 512], F32R, tag="pT", bufs=2)
                        for qt4 in range(4):
                            qi = qc * 4 + qt4
                            qs = slice(qi * 128, (qi + 1) * 128)

                            s = s_psum.tile([128, L], F32, tag="s")
                            for kc in range(2):
                                nc.tensor.matmul(
                                    s[:, kc * 512 : (kc + 1) * 512],
                                    qtp[:, h, qs],
                                    ktp[:, h, kc * 512 : (kc + 1) * 512],
                                    start=True,
                                    stop=True,
                                )
                            a = work.tile([128, L], F32R, tag="a", bufs=8)
                            nc.vector.tensor_add(a, s, zsb[:, qi, :])
                            rs = stats.tile([128, 1], F32, tag="rs", bufs=12)
                            nc.scalar.activation(
                                a,
                                a,
                                mybir.ActivationFunctionType.Exp,
                                scale=INV_SQRT_D,
                                accum_out=rs,
                            )
                            rec = stats.tile([128, 1], F32, tag="rec", bufs=12)
                            nc.vector.reciprocal(rec, rs)
                            nc.vector.tensor_scalar_mul(a, a, rec)
                            nc.sync.dma_start(out=sim[h, qs, :], in_=a.bitcast(F32))

                            # P^T via PE transpose, batched 4 tiles per PSUM bank
                            for half in range(2):
                                tp = t_psum.tile([128, 512], F32R, tag="tp")
                                for j in range(4):
                                    kb = half * 4 + j
                                    nc.tensor.transpose(
                                        tp[:, j * 128 : (j + 1) * 128],
                                        a[:, kb * 128 : (kb + 1) * 128],
                                        ident,
                                    )
                                dst = pT[
                                    :, half * 4 : half * 4 + 4, qt4 * 128 : (qt4 + 1) * 128
                                ]
                                src = tp.rearrange("p (t x) -> p t x", t=4)
                                nc.scalar.copy(dst, src)

                        # PV: out^T[j, q] accumulated over kk blocks (N=512)
                        ot = o_psum.tile([128, 512], F32, tag="ot")
                        for kb in range(8):
                            nc.tensor.matmul(
                                ot,
                                vsb[:, kb, h * 128 : (h + 1) * 128],
                                pT[:, kb, :],
                                start=(kb == 0),
                                stop=(kb == 7),
                            )
                        ots = work.tile([128, 512], F32, tag="ots", bufs=4)
                        nc.scalar.copy(ots, ot)
                        nc.sync.dma_start(
                            out=outT[h * 128 : (h + 1) * 128, qc * 512 : (qc + 1) * 512],
                            in_=ots,
                        )
                zpool.release()
    nc.finalize()
    return nc


_NC_CACHE = None


def _get_nc():
    global _NC_CACHE
    if _NC_CACHE is None:
        _NC_CACHE = build_nc()
    return _NC_CACHE


def kernel(query, keys, mask, relative_embedding, Wq, Wk, Wv, _trace=False, _nc=None):
    query = np.asarray(query, dtype=np.float32)
    keys = np.asarray(keys, dtype=np.float32)
    mask = np.asarray(mask, dtype=np.int32)
    rel = np.asarray(relative_embedding, dtype=np.float32)
    wqT = np.ascontiguousarray(np.asarray(Wq, dtype=np.float32).T)
    wkT = np.ascontiguousarray(np.asarray(Wk, dtype=np.float32).T)
    wvT = np.ascontiguousarray(np.asarray(Wv, dtype=np.float32).T)

    nc = _nc if _nc is not None else _get_nc()

    in_maps = []
    for b in range(B):
        in_maps.append(
            {
                "qT": np.ascontiguousarray(query[b].T),
                "kT": np.ascontiguousarray(keys[b].T),
                "msk": np.ascontiguousarray(mask[b]),
                "rel": np.ascontiguousarray(rel[b]),
                "wqT": wqT,
                "wkT": wkT,
                "wvT": wvT,
            }
        )

    res = run_bass_kernel_spmd(
        nc,
        in_maps,
        core_ids=list(range(B)),
        trace=_trace,
    )

    output = np.empty((B, L, D), np.float32)
    similarity = np.empty((H * B, L, L), np.float32)
    for b in range(B):
        output[b] = res.results[b]["outT"].T
        similarity[b::B] = res.results[b]["sim"]

    if _trace:
        kernel._last_results = res
    return output, similarity
